# revision 15
# baseline (speedup 1.0000x reference)
# Bass/Trainium2 kernel for nn_DictField (embedding_lookup):
# bilinear grid-sample from 7 small tables (coeff 144ch@64x64 + 6 basis
# grids), feats = basis_concat * coeff, then MLP 144->256->256->2 over
# 235520 points, data-parallel over 8 NeuronCores.
#
# Per-core plan (29440 points = 128 partitions x 230 columns):
#  - host pre-packs each table into per-cell quad rows, CORNER-INNERMOST
#    [c0:(00,01,10,11), c1:(...), ...] so the bilinear weighted sum runs
#    in the DVE 2x fast mode (contiguous 4-corner last dim).
#  - on device: bulk index/weight math in [128, 230] layout, int16 cell
#    ids in the SWDGE wrapped-16 index layout, dma_gather per chunk,
#    weighted corner reduction on DVE (1 fast mult + 1 fast pair-add +
#    1 strided add), feats -> channel-major via XBAR DMA transpose
#    (16x128 tiles), then an all-bf16 MLP on the PE with fp32 psum.
#    Output is written channel-major (2, 29440) per core; host gathers.
import os

import numpy as np

IM_H, IM_W = 640, 368
N_POINTS = IM_H * IM_W            # 235520
N_CORES = 8
NPC = N_POINTS // N_CORES         # 29440 per core
P = 128
G = NPC // P                      # 230 columns
BASIS_DIMS = [32, 32, 32, 16, 16, 16]
BASIS_RESOS = [32, 51, 70, 89, 108, 128]
SUM_DIMS = 144
HIDDEN = 256
OUT_DIM = 2
BBOX1 = (640.0, 368.0)

DT_NAME = os.environ.get("DICT_DT", "bfloat16")   # lerp-stage dtype knob
SKIP = os.environ.get("DICT_SKIP", "")            # debug: "mlp" | "lerp"
FC_DEFAULT = int(os.environ.get("DICT_FC", "8"))
NSUB = 512


def _pack_quad(table, clamp_border):
    """table (C, R, R) -> rows[(cells), C*4] quad-packed, corner-INNERMOST."""
    C, R, _ = table.shape
    if clamp_border:
        ys0 = np.arange(R); xs0 = np.arange(R)
        ys1 = np.minimum(ys0 + 1, R - 1); xs1 = np.minimum(xs0 + 1, R - 1)
    else:
        ys0 = np.arange(R - 1); xs0 = np.arange(R - 1)
        ys1 = ys0 + 1; xs1 = xs0 + 1
    c00 = table[:, ys0][:, :, xs0]
    c01 = table[:, ys0][:, :, xs1]
    c10 = table[:, ys1][:, :, xs0]
    c11 = table[:, ys1][:, :, xs1]
    quad = np.stack([c00, c01, c10, c11], axis=0)       # (4, C, H', W')
    quad = np.transpose(quad, (2, 3, 1, 0))             # (H', W', C, 4)
    ncell = quad.shape[0] * quad.shape[1]
    return np.ascontiguousarray(quad.reshape(ncell, 4 * C))


def _pad_rows_to_256b(rows, dt):
    itemsize = np.dtype(dt).itemsize
    pad = (-(rows.shape[1] * itemsize)) % 256
    if pad == 0:
        return np.ascontiguousarray(rows)
    out = np.zeros((rows.shape[0], rows.shape[1] + pad // itemsize), dt)
    out[:, : rows.shape[1]] = rows
    return out


def _chunks(fc):
    out = []
    g0 = 0
    while g0 < G:
        out.append((g0, min(fc, G - g0)))
        g0 += fc
    return out


def build_kernel(nc, dt_lerp, has_b0, has_b1, table_elems, fc):
    import concourse.mybir as mybir
    from concourse.tile import TileContext
    from concourse import masks

    F32 = mybir.dt.float32
    I16 = mybir.dt.int16
    I32 = mybir.dt.int32
    ALU = mybir.AluOpType
    ACTF = mybir.ActivationFunctionType
    DTL = {"float32": F32, "bfloat16": mybir.dt.bfloat16}[dt_lerp]

    coords = nc.dram_tensor("coords", (NPC, 2), F32, kind="ExternalInput")
    tabs = {}
    for name, (nrows, elem) in table_elems.items():
        tabs[name] = nc.dram_tensor(name, (nrows, elem), DTL,
                                    kind="ExternalInput")
    wdr = {}
    for nm, shape in (("w0a", (128, HIDDEN)), ("w0b", (16, HIDDEN)),
                      ("w1a", (128, HIDDEN)), ("w1b", (128, HIDDEN)),
                      ("w2a", (128, OUT_DIM)), ("w2b", (128, OUT_DIM))):
        wdr[nm] = nc.dram_tensor(nm, shape, DTL, kind="ExternalInput")
    if has_b0:
        b0r = nc.dram_tensor("b0r", (1, HIDDEN), DTL, kind="ExternalInput")
    if has_b1:
        b1r = nc.dram_tensor("b1r", (1, HIDDEN), DTL, kind="ExternalInput")
    outT = nc.dram_tensor("outT", (OUT_DIM, NPC), F32, kind="ExternalOutput")

    # (name, cellW, x0max, y0max, C, elem, is_coeff, reso)
    grids = [("coeff", 64, 63, 63, SUM_DIMS, table_elems["coeff"][1],
              True, None)]
    for i, (bd, reso) in enumerate(zip(BASIS_DIMS, BASIS_RESOS)):
        nm = f"basis{i}"
        grids.append((nm, reso - 1, reso - 2, reso - 2, bd,
                      table_elems[nm][1], False, reso))

    with TileContext(nc) as tc:
        W4s, Is = {}, {}
        with tc.tile_pool(name="persist", bufs=1) as pp:
            ws = {}
            for nm, shape in (("w0a", (128, HIDDEN)), ("w0b", (16, HIDDEN)),
                              ("w1a", (128, HIDDEN)), ("w1b", (128, HIDDEN)),
                              ("w2a", (128, OUT_DIM)), ("w2b", (128, OUT_DIM))):
                t = pp.tile(list(shape), DTL, tag=nm)
                nc.sync.dma_start(out=t[:], in_=wdr[nm][:, :])
                ws[nm] = t
            if has_b0 or has_b1:
                ones = pp.tile([1, NSUB], DTL)
                nc.vector.memset(ones[:], 1.0)
            if has_b0:
                b0s = pp.tile([1, HIDDEN], DTL)
                nc.sync.dma_start(out=b0s[:], in_=b0r[:, :])
            if has_b1:
                b1s = pp.tile([1, HIDDEN], DTL)
                nc.sync.dma_start(out=b1s[:], in_=b1r[:, :])

            with tc.tile_pool(name="prep", bufs=1) as prep:
                csb = prep.tile([P, 2 * G], F32)
                nc.sync.dma_start(out=csb[:], in_=coords[:, :].rearrange(
                    "(q f) c -> q (f c)", q=P))
                xq = prep.tile([P, G], F32)
                yq = prep.tile([P, G], F32)
                cv = csb[:, :].rearrange("p (f c) -> p f c", c=2)
                nc.vector.tensor_copy(xq[:], cv[:, :, 0])
                nc.vector.tensor_copy(yq[:], cv[:, :, 1])

                def ts(out, in0, s1, s2, op0, op1=None):
                    if op1 is None:
                        nc.vector.tensor_scalar(out, in0, s1, None, op0)
                    else:
                        nc.vector.tensor_scalar(out, in0, s1, s2, op0, op1)

                def tt(out, a, b, op):
                    nc.vector.tensor_tensor(out=out, in0=a, in1=b, op=op)

                f1 = prep.tile([P, G], F32)
                f2 = prep.tile([P, G], F32)
                ta = prep.tile([P, G], F32)
                tb = prep.tile([P, G], I32)

                def floor_weights(ix, x0max, x0f, wx):
                    ts(ta[:], ix[:], 0.5, None, ALU.subtract)
                    nc.vector.tensor_copy(tb[:], ta[:])      # f32->i32 (RNE)
                    nc.vector.tensor_copy(x0f[:], tb[:])     # i32->f32
                    tt(ta[:], ix[:], x0f[:], ALU.subtract)
                    ts(ta[:], ta[:], 1.0, None, ALU.is_ge)
                    tt(x0f[:], x0f[:], ta[:], ALU.add)
                    ts(x0f[:], x0f[:], float(x0max), None, ALU.min)
                    ts(x0f[:], x0f[:], 0.0, None, ALU.max)
                    tt(wx[:], ix[:], x0f[:], ALU.subtract)

                for gi, (nm, cellW, x0max, y0max, C, elem, is_coeff, reso) \
                        in enumerate(grids):
                    x0f = prep.tile([P, G], F32, tag="x0f")
                    y0f = prep.tile([P, G], F32, tag="y0f")
                    wx = prep.tile([P, G], F32, tag="wx")
                    wy = prep.tile([P, G], F32, tag="wy")
                    for (cq, axis, ix_, w_, z0f, zmax) in (
                            (xq, 0, f1, wx, x0f, x0max),
                            (yq, 1, f2, wy, y0f, y0max)):
                        if is_coeff:
                            s = np.float32(2.0) / np.float32(BBOX1[axis])
                            ts(ix_[:], cq[:], float(s), 1.0,
                               ALU.mult, ALU.subtract)        # norm
                            ts(ix_[:], ix_[:], 1.0, None, ALU.add)
                            ts(ix_[:], ix_[:], 64.0, 1.0,
                               ALU.mult, ALU.subtract)
                            ts(ix_[:], ix_[:], 0.5, 0.0, ALU.mult, ALU.max)
                            ts(ix_[:], ix_[:], 63.0, None, ALU.min)
                        else:
                            inv = np.float32(1.0) / np.float32(reso)
                            ts(ta[:], cq[:], float(inv), None, ALU.mult)
                            nc.vector.tensor_copy(tb[:], ta[:])
                            nc.vector.tensor_copy(ta[:], tb[:])
                            ts(ta[:], ta[:], float(reso), None, ALU.mult)
                            tt(ix_[:], cq[:], ta[:], ALU.subtract)
                            ts(ta[:], ix_[:], 0.0, float(reso),
                               ALU.is_lt, ALU.mult)
                            tt(ix_[:], ix_[:], ta[:], ALU.add)
                            ts(ta[:], ix_[:], float(reso), float(reso),
                               ALU.is_ge, ALU.mult)
                            tt(ix_[:], ix_[:], ta[:], ALU.subtract)  # fmod
                            inv2 = np.float32(1.0) / np.float32(reso * 0.5)
                            ts(ix_[:], ix_[:], float(inv2), 1.0,
                               ALU.mult, ALU.subtract)        # loc
                            ts(ix_[:], ix_[:], 1.0, None, ALU.add)
                            ts(ix_[:], ix_[:], 0.5, float(reso - 1),
                               ALU.mult, ALU.mult)
                        floor_weights(ix_, zmax, z0f, w_)

                    ts(f1[:], y0f[:], float(cellW), None, ALU.mult)
                    tt(f1[:], f1[:], x0f[:], ALU.add)
                    ci = prep.tile([P, G], I16, tag="ci")
                    nc.vector.tensor_copy(ci[:], f1[:])
                    Iw = pp.tile([P, 8 * G], I16, tag=f"I{gi}")
                    cm = prep.tile([16, 8 * G], I16, tag="cm")
                    for k in range(8):
                        nc.sync.dma_start(out=cm[0:16, k * G:(k + 1) * G],
                                          in_=ci[16 * k:16 * (k + 1), :])
                    nc.vector.tensor_copy(
                        Iw[0:16, :].rearrange("p (g k) -> p g k", k=8),
                        cm[0:16, :].rearrange("p (k g) -> p k g", k=8)
                        .transpose([0, 2, 1]))
                    nc.sync.dma_start(out=Iw[16:32, :], in_=Iw[0:16, :])
                    nc.sync.dma_start(out=Iw[32:64, :], in_=Iw[0:32, :])
                    nc.sync.dma_start(out=Iw[64:128, :], in_=Iw[0:64, :])
                    Is[nm] = Iw

                    W4f = prep.tile([P, G, 4], F32, tag="W4f")
                    v = W4f[:, :, :]
                    tt(v[:, :, 3], wx[:], wy[:], ALU.mult)
                    tt(v[:, :, 1], wx[:], v[:, :, 3], ALU.subtract)
                    tt(v[:, :, 2], wy[:], v[:, :, 3], ALU.subtract)
                    ts(f2[:], wx[:], -1.0, 1.0, ALU.mult, ALU.add)
                    tt(v[:, :, 0], f2[:], v[:, :, 2], ALU.subtract)
                    W4 = pp.tile([P, G, 4], DTL, tag=f"W4{gi}")
                    nc.vector.tensor_copy(W4[:, :, :], W4f[:, :, :])
                    W4s[nm] = W4

            # ---- chunk loop ----
            with (
                tc.tile_pool(name="gath", bufs=2) as gp,
                tc.tile_pool(name="mid", bufs=2) as mp,
                tc.tile_pool(name="mlp", bufs=2) as lp,
                tc.tile_pool(name="psA", bufs=1, space="PSUM") as psA,
                tc.tile_pool(name="psB", bufs=2, space="PSUM") as psB,
            ):
                for (g0, fci) in _chunks(fc):
                    NCH = fci * P
                    gbuf = mp.tile([P, fc, SUM_DIMS], DTL, tag="gbuf")
                    cbuf = mp.tile([P, fc, SUM_DIMS], DTL, tag="cbuf")
                    off = 0
                    for nm, cellW, x0max, y0max, C, elem, is_coeff, reso \
                            in grids:
                        Q = gp.tile([P, fc, elem], DTL, tag=f"q_{nm}")
                        nc.gpsimd.dma_gather(
                            out_ap=Q[:, 0:fci, :],
                            in_ap=tabs[nm][:, :],
                            idxs_ap=Is[nm][:, 8 * g0: 8 * (g0 + fci)],
                            num_idxs=NCH,
                            num_idxs_reg=NCH,
                            elem_size=elem,
                        )
                        # corner-innermost view of the real payload
                        Qv = Q[:, 0:fci, 0:4 * C].rearrange(
                            "p g (c f) -> p g c f", f=4)
                        w4 = W4s[nm][:, g0:g0 + fci, :]
                        nc.vector.tensor_tensor(
                            out=Qv, in0=Qv,
                            in1=w4.unsqueeze(2).broadcast_to((P, fci, C, 4)),
                            op=ALU.mult)
                        nc.vector.tensor_tensor(
                            out=Qv[:, :, :, 0:2], in0=Qv[:, :, :, 0:2],
                            in1=Qv[:, :, :, 2:4], op=ALU.add)
                        dst = cbuf if is_coeff else gbuf
                        o0 = 0 if is_coeff else off
                        nc.vector.tensor_tensor(
                            out=dst[:, 0:fci, o0:o0 + C],
                            in0=Qv[:, :, :, 0], in1=Qv[:, :, :, 1],
                            op=ALU.add)
                        if not is_coeff:
                            off += C
                    # feats has one ghost g-column so the tail XBAR's
                    # [128,128] source window never runs off the tile
                    feats = mp.tile([P, fc + 1, SUM_DIMS], DTL, tag="feats")
                    nc.vector.tensor_tensor(
                        out=feats[:, 0:fci, :], in0=gbuf[:, 0:fci, :],
                        in1=cbuf[:, 0:fci, :], op=ALU.mult)
                    if SKIP == "mlp":
                        fr = lp.tile([P, fc], F32, tag="fr")
                        import concourse.mybir as _mb
                        nc.vector.tensor_reduce(
                            fr[:, 0:fci], feats[:, 0:fci, :],
                            axis=_mb.AxisListType.X, op=ALU.add)
                        nc.sync.dma_start(
                            out=outT[:, :].flatten()[0:NPC].rearrange(
                                "(p g) -> p g", p=P)[:, g0:g0 + fci],
                            in_=fr[:, 0:fci])
                        continue

                    # XBAR transposes: per g-block, channels 0:128 into fTa
                    # and a [128,128] window at channel 128 into fTbx
                    # (rows 0:16 real, rest ghost).
                    fTa = lp.tile([128, fc * P], DTL, tag="fTa")
                    fTbx = lp.tile([128, fc * P], DTL, tag="fTbx")
                    ff = feats[:, :, :].rearrange("p g c -> p (g c)")
                    for j in range(fci):
                        nc.sync.dma_start(
                            out=fTa[:, j * P:(j + 1) * P],
                            in_=ff[:, j * SUM_DIMS:j * SUM_DIMS + 128],
                            transpose=True)
                        nc.sync.dma_start(
                            out=fTbx[:, j * P:(j + 1) * P],
                            in_=ff[:, j * SUM_DIMS + 128:
                                   j * SUM_DIMS + 256],
                            transpose=True)

                    nsp = (NCH + NSUB - 1) // NSUB

                    def layer(movs, wa, wb, hout, act, extra=None):
                        """hout[m][:, n] = act(wa/wb.T @ movs), k-reuse."""
                        for m in range(2):
                            pss = [psA.tile([128, NSUB], F32,
                                            tag=f"ps{m}_{i}",
                                            name=f"ps{m}_{i}")
                                   for i in range(nsp)]
                            for ki, (wk, mv) in enumerate(
                                    ((wa, movs[0]), (wb, movs[1]))):
                                last = ki == 1 and extra is None
                                for i in range(nsp):
                                    n0 = i * NSUB
                                    n1 = min(n0 + NSUB, NCH)
                                    nc.tensor.matmul(
                                        pss[i][:, 0:n1 - n0],
                                        wk[:, m * 128:(m + 1) * 128],
                                        mv[:, n0:n1],
                                        start=(ki == 0), stop=last)
                            if extra is not None:
                                bsrc, msrc = extra
                                for i in range(nsp):
                                    n0 = i * NSUB
                                    n1 = min(n0 + NSUB, NCH)
                                    nc.tensor.matmul(
                                        pss[i][:, 0:n1 - n0],
                                        bsrc[:, m * 128:(m + 1) * 128],
                                        msrc[:, 0:n1 - n0],
                                        start=False, stop=True)
                            for i in range(nsp):
                                n0 = i * NSUB
                                n1 = min(n0 + NSUB, NCH)
                                act(hout[m][:, n0:n1], pss[i][:, 0:n1 - n0])

                    h0 = (lp.tile([128, fc * P], DTL, tag="h0a", name="h0a"),
                          lp.tile([128, fc * P], DTL, tag="h0b", name="h0b"))
                    layer((fTa, fTbx[0:16, :]), ws["w0a"], ws["w0b"], h0,
                          lambda o, i: nc.scalar.activation(o, i, ACTF.Relu),
                          extra=(b0s, ones) if has_b0 else None)
                    h1 = (lp.tile([128, fc * P], DTL, tag="h1a", name="h1a"),
                          lp.tile([128, fc * P], DTL, tag="h1b", name="h1b"))
                    layer(h0, ws["w1a"], ws["w1b"], h1,
                          lambda o, i: nc.scalar.activation(o, i, ACTF.Relu),
                          extra=(b1s, ones) if has_b1 else None)

                    stage = lp.tile([OUT_DIM, fc * P], F32, tag="stage")
                    for i in range(nsp):
                        n0 = i * NSUB
                        n1 = min(n0 + NSUB, NCH)
                        ps = psB.tile([OUT_DIM, NSUB], F32, tag="pso")
                        pss = ps[:, 0:n1 - n0]
                        nc.tensor.matmul(pss, ws["w2a"][:, :],
                                         h1[0][:, n0:n1],
                                         start=True, stop=False)
                        nc.tensor.matmul(pss, ws["w2b"][:, :],
                                         h1[1][:, n0:n1],
                                         start=False, stop=True)
                        nc.scalar.copy(stage[:, n0:n1], pss)
                    nc.sync.dma_start(out=outT[:, g0 * P:(g0 + fci) * P],
                                      in_=stage[:, 0:NCH])
    return nc


_CACHE = {}


def _get_compiled(dt_lerp, has_b0, has_b1, table_elems, fc):
    key = (dt_lerp, has_b0, has_b1, fc)
    if key in _CACHE:
        return _CACHE[key]
    import concourse.bacc as bacc
    nc = bacc.Bacc("TRN2", target_bir_lowering=False,
                   dynamic_dma_scratch_size=int(
                       os.environ.get("DICT_SCRATCH", "16384")))
    build_kernel(nc, dt_lerp, has_b0, has_b1, table_elems, fc)
    nc.compile()
    _CACHE[key] = nc
    return nc


def kernel(coordinates, coeffs, basis_0, basis_1, basis_2, basis_3, basis_4,
           basis_5, w0, b0, w1, b1, w2):
    import ml_dtypes
    from concourse.bass_utils import run_bass_kernel_spmd

    dt_lerp = DT_NAME
    np_dt = np.float32 if dt_lerp == "float32" else ml_dtypes.bfloat16
    fc = FC_DEFAULT

    coordinates = np.ascontiguousarray(np.asarray(coordinates, np.float32))
    packed, table_elems = {}, {}
    pc = _pack_quad(np.asarray(coeffs, np.float32)[0], clamp_border=True)
    pc = _pad_rows_to_256b(pc.astype(np_dt), np_dt)
    packed["coeff"] = pc
    table_elems["coeff"] = (pc.shape[0], pc.shape[1])
    for i, b in enumerate([basis_0, basis_1, basis_2, basis_3, basis_4,
                           basis_5]):
        pb = _pack_quad(np.asarray(b, np.float32)[0], clamp_border=False)
        pb = _pad_rows_to_256b(pb.astype(np_dt), np_dt)
        packed[f"basis{i}"] = pb
        table_elems[f"basis{i}"] = (pb.shape[0], pb.shape[1])

    w0 = np.asarray(w0, np.float32); w1 = np.asarray(w1, np.float32)
    w2 = np.asarray(w2, np.float32)
    b0 = np.asarray(b0, np.float32); b1 = np.asarray(b1, np.float32)
    has_b0 = bool(np.any(b0)); has_b1 = bool(np.any(b1))

    nc = _get_compiled(dt_lerp, has_b0, has_b1, table_elems, fc)

    shared = {
        "w0a": np.ascontiguousarray(w0[0:128]).astype(np_dt),
        "w0b": np.ascontiguousarray(w0[128:144]).astype(np_dt),
        "w1a": np.ascontiguousarray(w1[0:128]).astype(np_dt),
        "w1b": np.ascontiguousarray(w1[128:256]).astype(np_dt),
        "w2a": np.ascontiguousarray(w2[0:128]).astype(np_dt),
        "w2b": np.ascontiguousarray(w2[128:256]).astype(np_dt),
    }
    shared.update(packed)
    if has_b0:
        shared["b0r"] = np.ascontiguousarray(b0.reshape(1, HIDDEN))
    if has_b1:
        shared["b1r"] = np.ascontiguousarray(b1.reshape(1, HIDDEN))

    in_maps = []
    for c in range(N_CORES):
        m = dict(shared)
        m["coords"] = np.ascontiguousarray(coordinates[c * NPC:(c + 1) * NPC])
        in_maps.append(m)

    res = run_bass_kernel_spmd(
        nc, in_maps, core_ids=list(range(N_CORES)),
        trace=bool(int(os.environ.get("DICT_TRACE", "0"))))
    kernel.last_results = res
    # device stores column (g*128 + p) = point (p*230 + g); unscramble here
    outs = []
    for c in range(N_CORES):
        o = res.results[c]["outT"].reshape(OUT_DIM, G, P)
        outs.append(np.transpose(o, (0, 2, 1)).reshape(OUT_DIM, NPC))
    full = np.concatenate(outs, axis=1)                      # (2, N)
    return np.ascontiguousarray(full.T).reshape(IM_H, IM_W, OUT_DIM)


# revision 24
# speedup vs baseline: 1.5974x; 1.5974x over previous
# Bass/Trainium2 kernel for nn_DictField (embedding_lookup):
# bilinear grid-sample from 7 small tables (coeff 144ch@64x64 + 6 basis
# grids), feats = basis_concat * coeff, then MLP 144->256->256->2 over
# 235520 points, data-parallel over 8 NeuronCores.
#
# Per-core plan (29440 points = 128 partitions x 230 columns):
#  - host pre-packs each table into per-cell quad rows, CORNER-INNERMOST
#    [c0:(00,01,10,11), c1:(...), ...] so the bilinear weighted sum runs
#    in the DVE 2x fast mode (contiguous 4-corner last dim).
#  - on device: bulk index/weight math in [128, 230] layout, int16 cell
#    ids in the SWDGE wrapped-16 index layout, dma_gather per chunk,
#    weighted corner reduction on DVE (1 fast mult + 1 fast pair-add +
#    1 strided add), feats -> channel-major via XBAR DMA transpose
#    (16x128 tiles), then an all-bf16 MLP on the PE with fp32 psum.
#    Output is written channel-major (2, 29440) per core; host gathers.
import os

import numpy as np

IM_H, IM_W = 640, 368
N_POINTS = IM_H * IM_W            # 235520
N_CORES = 8
NPC = N_POINTS // N_CORES         # 29440 per core
P = 128
G = NPC // P                      # 230 columns
BASIS_DIMS = [32, 32, 32, 16, 16, 16]
BASIS_RESOS = [32, 51, 70, 89, 108, 128]
SUM_DIMS = 144
HIDDEN = 256
OUT_DIM = 2
BBOX1 = (640.0, 368.0)

DT_NAME = os.environ.get("DICT_DT", "bfloat16")   # lerp-stage dtype knob
SKIP = os.environ.get("DICT_SKIP", "")            # debug: "mlp" | "lerp"
FC_DEFAULT = int(os.environ.get("DICT_FC", "8"))
NSUB = 512


def _pack_quad(table, clamp_border):
    """table (C, R, R) -> rows[(cells), C*4] quad-packed, corner-INNERMOST."""
    C, R, _ = table.shape
    if clamp_border:
        ys0 = np.arange(R); xs0 = np.arange(R)
        ys1 = np.minimum(ys0 + 1, R - 1); xs1 = np.minimum(xs0 + 1, R - 1)
    else:
        ys0 = np.arange(R - 1); xs0 = np.arange(R - 1)
        ys1 = ys0 + 1; xs1 = xs0 + 1
    c00 = table[:, ys0][:, :, xs0]
    c01 = table[:, ys0][:, :, xs1]
    c10 = table[:, ys1][:, :, xs0]
    c11 = table[:, ys1][:, :, xs1]
    quad = np.stack([c00, c01, c10, c11], axis=0)       # (4, C, H', W')
    quad = np.transpose(quad, (2, 3, 1, 0))             # (H', W', C, 4)
    ncell = quad.shape[0] * quad.shape[1]
    return np.ascontiguousarray(quad.reshape(ncell, 4 * C))


def _pad_rows_to_256b(rows, dt):
    itemsize = np.dtype(dt).itemsize
    pad = (-(rows.shape[1] * itemsize)) % 256
    if pad == 0:
        return np.ascontiguousarray(rows)
    out = np.zeros((rows.shape[0], rows.shape[1] + pad // itemsize), dt)
    out[:, : rows.shape[1]] = rows
    return out


def _chunks(fc):
    out = []
    g0 = 0
    while g0 < G:
        out.append((g0, min(fc, G - g0)))
        g0 += fc
    return out


def build_kernel(nc, dt_lerp, has_b0, has_b1, table_elems, fc):
    import concourse.mybir as mybir
    from concourse.tile import TileContext
    from concourse import masks

    F32 = mybir.dt.float32
    I16 = mybir.dt.int16
    I32 = mybir.dt.int32
    ALU = mybir.AluOpType
    ACTF = mybir.ActivationFunctionType
    DTL = {"float32": F32, "bfloat16": mybir.dt.bfloat16}[dt_lerp]

    coords = nc.dram_tensor("coords", (NPC, 2), F32, kind="ExternalInput")
    tabs = {}
    for name, (nrows, elem) in table_elems.items():
        tabs[name] = nc.dram_tensor(name, (nrows, elem), DTL,
                                    kind="ExternalInput")
    wdr = {}
    for nm, shape in (("w0a", (128, HIDDEN)), ("w0b", (16, HIDDEN)),
                      ("w1a", (128, HIDDEN)), ("w1b", (128, HIDDEN)),
                      ("w2a", (128, OUT_DIM)), ("w2b", (128, OUT_DIM))):
        wdr[nm] = nc.dram_tensor(nm, shape, DTL, kind="ExternalInput")
    if has_b0:
        b0r = nc.dram_tensor("b0r", (P, 2), F32, kind="ExternalInput")
    if has_b1:
        b1r = nc.dram_tensor("b1r", (P, 2), F32, kind="ExternalInput")
    outT = nc.dram_tensor("outT", (OUT_DIM, NPC), F32, kind="ExternalOutput")

    # (name, cellW, x0max, y0max, C, elem, is_coeff, reso)
    grids = [("coeff", 64, 63, 63, SUM_DIMS, table_elems["coeff"][1],
              True, None)]
    for i, (bd, reso) in enumerate(zip(BASIS_DIMS, BASIS_RESOS)):
        nm = f"basis{i}"
        grids.append((nm, reso - 1, reso - 2, reso - 2, bd,
                      table_elems[nm][1], False, reso))

    with TileContext(nc) as tc:
        W4s, Is = {}, {}
        with tc.tile_pool(name="persist", bufs=1) as pp:
            identl = pp.tile([P, P], DTL)
            masks.make_identity(nc, identl[:])
            ws = {}
            for nm, shape in (("w0a", (128, HIDDEN)), ("w0b", (16, HIDDEN)),
                              ("w1a", (128, HIDDEN)), ("w1b", (128, HIDDEN)),
                              ("w2a", (128, OUT_DIM)), ("w2b", (128, OUT_DIM))):
                t = pp.tile(list(shape), DTL, tag=nm)
                nc.sync.dma_start(out=t[:], in_=wdr[nm][:, :])
                ws[nm] = t
            b0s = b1s = None
            if has_b0:
                b0s = pp.tile([P, 2], F32)
                nc.sync.dma_start(out=b0s[:], in_=b0r[:, :])
            if has_b1:
                b1s = pp.tile([P, 2], F32)
                nc.sync.dma_start(out=b1s[:], in_=b1r[:, :])

            with tc.tile_pool(name="prep", bufs=1) as prep:
                csb = prep.tile([P, 2 * G], F32)
                nc.sync.dma_start(out=csb[:], in_=coords[:, :].rearrange(
                    "(q f) c -> q (f c)", q=P))
                xq = prep.tile([P, G], F32)
                yq = prep.tile([P, G], F32)
                cv = csb[:, :].rearrange("p (f c) -> p f c", c=2)
                nc.vector.tensor_copy(xq[:], cv[:, :, 0])
                nc.vector.tensor_copy(yq[:], cv[:, :, 1])

                def ts(out, in0, s1, s2, op0, op1=None):
                    if op1 is None:
                        nc.vector.tensor_scalar(out, in0, s1, None, op0)
                    else:
                        nc.vector.tensor_scalar(out, in0, s1, s2, op0, op1)

                def tt(out, a, b, op):
                    nc.vector.tensor_tensor(out=out, in0=a, in1=b, op=op)

                f1 = prep.tile([P, G], F32)
                f2 = prep.tile([P, G], F32)
                ta = prep.tile([P, G], F32)
                tb = prep.tile([P, G], I32)

                def floor_weights(ix, x0max, x0f, wx):
                    ts(ta[:], ix[:], 0.5, None, ALU.subtract)
                    nc.vector.tensor_copy(tb[:], ta[:])      # f32->i32 (RNE)
                    nc.vector.tensor_copy(x0f[:], tb[:])     # i32->f32
                    tt(ta[:], ix[:], x0f[:], ALU.subtract)
                    ts(ta[:], ta[:], 1.0, None, ALU.is_ge)
                    tt(x0f[:], x0f[:], ta[:], ALU.add)
                    ts(x0f[:], x0f[:], float(x0max), None, ALU.min)
                    ts(x0f[:], x0f[:], 0.0, None, ALU.max)
                    tt(wx[:], ix[:], x0f[:], ALU.subtract)

                for gi, (nm, cellW, x0max, y0max, C, elem, is_coeff, reso) \
                        in enumerate(grids):
                    x0f = prep.tile([P, G], F32, tag="x0f")
                    y0f = prep.tile([P, G], F32, tag="y0f")
                    wx = prep.tile([P, G], F32, tag="wx")
                    wy = prep.tile([P, G], F32, tag="wy")
                    for (cq, axis, ix_, w_, z0f, zmax) in (
                            (xq, 0, f1, wx, x0f, x0max),
                            (yq, 1, f2, wy, y0f, y0max)):
                        if is_coeff:
                            s = np.float32(2.0) / np.float32(BBOX1[axis])
                            ts(ix_[:], cq[:], float(s), 1.0,
                               ALU.mult, ALU.subtract)        # norm
                            ts(ix_[:], ix_[:], 1.0, None, ALU.add)
                            ts(ix_[:], ix_[:], 64.0, 1.0,
                               ALU.mult, ALU.subtract)
                            ts(ix_[:], ix_[:], 0.5, 0.0, ALU.mult, ALU.max)
                            ts(ix_[:], ix_[:], 63.0, None, ALU.min)
                        else:
                            inv = np.float32(1.0) / np.float32(reso)
                            ts(ta[:], cq[:], float(inv), None, ALU.mult)
                            nc.vector.tensor_copy(tb[:], ta[:])
                            nc.vector.tensor_copy(ta[:], tb[:])
                            ts(ta[:], ta[:], float(reso), None, ALU.mult)
                            tt(ix_[:], cq[:], ta[:], ALU.subtract)
                            ts(ta[:], ix_[:], 0.0, float(reso),
                               ALU.is_lt, ALU.mult)
                            tt(ix_[:], ix_[:], ta[:], ALU.add)
                            ts(ta[:], ix_[:], float(reso), float(reso),
                               ALU.is_ge, ALU.mult)
                            tt(ix_[:], ix_[:], ta[:], ALU.subtract)  # fmod
                            inv2 = np.float32(1.0) / np.float32(reso * 0.5)
                            ts(ix_[:], ix_[:], float(inv2), 1.0,
                               ALU.mult, ALU.subtract)        # loc
                            ts(ix_[:], ix_[:], 1.0, None, ALU.add)
                            ts(ix_[:], ix_[:], 0.5, float(reso - 1),
                               ALU.mult, ALU.mult)
                        floor_weights(ix_, zmax, z0f, w_)

                    ts(f1[:], y0f[:], float(cellW), None, ALU.mult)
                    tt(f1[:], f1[:], x0f[:], ALU.add)
                    ci = prep.tile([P, G], I16, tag="ci")
                    nc.vector.tensor_copy(ci[:], f1[:])
                    Iw = pp.tile([P, 8 * G], I16, tag=f"I{gi}")
                    cm = prep.tile([16, 8 * G], I16, tag="cm")
                    for k in range(8):
                        nc.sync.dma_start(out=cm[0:16, k * G:(k + 1) * G],
                                          in_=ci[16 * k:16 * (k + 1), :])
                    nc.vector.tensor_copy(
                        Iw[0:16, :].rearrange("p (g k) -> p g k", k=8),
                        cm[0:16, :].rearrange("p (k g) -> p k g", k=8)
                        .transpose([0, 2, 1]))
                    nc.sync.dma_start(out=Iw[16:32, :], in_=Iw[0:16, :])
                    nc.sync.dma_start(out=Iw[32:64, :], in_=Iw[0:32, :])
                    nc.sync.dma_start(out=Iw[64:128, :], in_=Iw[0:64, :])
                    Is[nm] = Iw

                    W4f = prep.tile([P, G, 4], F32, tag="W4f")
                    v = W4f[:, :, :]
                    tt(v[:, :, 3], wx[:], wy[:], ALU.mult)
                    tt(v[:, :, 1], wx[:], v[:, :, 3], ALU.subtract)
                    tt(v[:, :, 2], wy[:], v[:, :, 3], ALU.subtract)
                    ts(f2[:], wx[:], -1.0, 1.0, ALU.mult, ALU.add)
                    tt(v[:, :, 0], f2[:], v[:, :, 2], ALU.subtract)
                    W4 = pp.tile([P, G, 4], DTL, tag=f"W4{gi}")
                    nc.vector.tensor_copy(W4[:, :, :], W4f[:, :, :])
                    W4s[nm] = W4

            # ---- chunk loop ----
            with (
                tc.tile_pool(name="gath", bufs=2) as gp,
                tc.tile_pool(name="mid", bufs=2) as mp,
                tc.tile_pool(name="mlp", bufs=2) as lp,
                tc.tile_pool(name="psA", bufs=1, space="PSUM") as psA,
                tc.tile_pool(name="psB", bufs=2, space="PSUM") as psB,
            ):
                for (g0, fci) in _chunks(fc):
                    NCH = fci * P
                    gbuf = mp.tile([P, fc, SUM_DIMS], DTL, tag="gbuf")
                    cbuf = mp.tile([P, fc, SUM_DIMS], DTL, tag="cbuf")
                    off = 0
                    for nm, cellW, x0max, y0max, C, elem, is_coeff, reso \
                            in grids:
                        Q = gp.tile([P, fc, elem], DTL, tag=f"q_{nm}")
                        nc.gpsimd.dma_gather(
                            out_ap=Q[:, 0:fci, :],
                            in_ap=tabs[nm][:, :],
                            idxs_ap=Is[nm][:, 8 * g0: 8 * (g0 + fci)],
                            num_idxs=NCH,
                            num_idxs_reg=NCH,
                            elem_size=elem,
                        )
                        # corner-innermost view of the real payload
                        Qv = Q[:, 0:fci, 0:4 * C].rearrange(
                            "p g (c f) -> p g c f", f=4)
                        w4 = W4s[nm][:, g0:g0 + fci, :]
                        nc.vector.tensor_tensor(
                            out=Qv, in0=Qv,
                            in1=w4.unsqueeze(2).broadcast_to((P, fci, C, 4)),
                            op=ALU.mult)
                        nc.vector.tensor_tensor(
                            out=Qv[:, :, :, 0:2], in0=Qv[:, :, :, 0:2],
                            in1=Qv[:, :, :, 2:4], op=ALU.add)
                        dst = cbuf if is_coeff else gbuf
                        o0 = 0 if is_coeff else off
                        nc.vector.tensor_tensor(
                            out=dst[:, 0:fci, o0:o0 + C],
                            in0=Qv[:, :, :, 0], in1=Qv[:, :, :, 1],
                            op=ALU.add)
                        if not is_coeff:
                            off += C
    # feats split into a contiguous 128-channel block (fA) and the
                    # 16-channel tail (fB) so each transposes in ONE XBAR.
                    fA = mp.tile([P, fc, 128], DTL, tag="fA")
                    fB = mp.tile([P, fc, 16], DTL, tag="fB")
                    nc.vector.tensor_tensor(
                        out=fA[:, 0:fci, :], in0=gbuf[:, 0:fci, 0:128],
                        in1=cbuf[:, 0:fci, 0:128], op=ALU.mult)
                    nc.vector.tensor_tensor(
                        out=fB[:, 0:fci, :], in0=gbuf[:, 0:fci, 128:144],
                        in1=cbuf[:, 0:fci, 128:144], op=ALU.mult)
                    if SKIP == "mlp":
                        fr = lp.tile([P, fc], F32, tag="fr")
                        import concourse.mybir as _mb
                        nc.vector.tensor_reduce(
                            fr[:, 0:fci], fA[:, 0:fci, :],
                            axis=_mb.AxisListType.X, op=ALU.add)
                        nc.sync.dma_start(
                            out=outT[:, :].flatten()[0:NPC].rearrange(
                                "(p g) -> p g", p=P)[:, g0:g0 + fci],
                            in_=fr[:, 0:fci])
                        continue

                    # main: one XBAR, fTa[c, j, p] = fA[p, j, c].
                    # tail: per-g PE transpose + copy (16 channels).
                    fTa = lp.tile([128, fc, P], DTL, tag="fTa")
                    fTb = lp.tile([16, fc * P], DTL, tag="fTb")
                    nc.sync.dma_start(
                        out=fTa[:, :, :],
                        in_=fA[:, :, :].rearrange("p a b -> p (a b)"),
                        transpose=True)
                    for j in range(fci):
                        pB = psB.tile([16, P], DTL, tag="pB")
                        nc.tensor.transpose(pB[:, :], fB[:, j, :],
                                            identl[:])
                        nc.scalar.copy(fTb[:, j * P:(j + 1) * P], pB[:, :])
                    fTav = fTa[:, :, :].rearrange("c a b -> c (a b)")

                    nsp = (NCH + NSUB - 1) // NSUB

                    def layer(mova, movb_blocked, wa, wb, hout, act):
                        """hout[m] = act(wa.T @ mova + wb.T @ movb).

                        movb_blocked: either a full [128, NCH] tile, or
                        ("blk16", tile) where block j's 16 moving rows sit
                        at partitions j*16 and cover columns j*128..+128.
                        """
                        for m in range(2):
                            pss = [psA.tile([128, NSUB], F32,
                                            tag=f"ps_{i}", name=f"ps_{i}")
                                   for i in range(nsp)]
                            for i in range(nsp):
                                n0 = i * NSUB
                                n1 = min(n0 + NSUB, NCH)
                                nc.tensor.matmul(
                                    pss[i][:, 0:n1 - n0],
                                    wa[:, m * 128:(m + 1) * 128],
                                    mova[:, n0:n1],
                                    start=True, stop=False)
                            for i in range(nsp):
                                n0 = i * NSUB
                                n1 = min(n0 + NSUB, NCH)
                                nc.tensor.matmul(
                                    pss[i][:, 0:n1 - n0],
                                    wb[:, m * 128:(m + 1) * 128],
                                    movb_blocked[:, n0:n1],
                                    start=False, stop=True)
                            for i in range(nsp):
                                n0 = i * NSUB
                                n1 = min(n0 + NSUB, NCH)
                                act(hout[m][:, n0:n1], pss[i][:, 0:n1 - n0],
                                    m)

                    def relu_b(bs):
                        def f(o, i, m):
                            if bs is None:
                                nc.scalar.activation(o, i, ACTF.Relu)
                            else:
                                nc.scalar.activation(o, i, ACTF.Relu,
                                                     bias=bs[:, m:m + 1])
                        return f

                    h0 = (lp.tile([128, fc * P], DTL, tag="h0a", name="h0a"),
                          lp.tile([128, fc * P], DTL, tag="h0b", name="h0b"))
                    layer(fTav, fTb, ws["w0a"], ws["w0b"], h0,
                          relu_b(b0s))
                    h1 = (lp.tile([128, fc * P], DTL, tag="h1a", name="h1a"),
                          lp.tile([128, fc * P], DTL, tag="h1b", name="h1b"))
                    layer(h0[0], h0[1], ws["w1a"], ws["w1b"], h1,
                          relu_b(b1s))

                    stage = lp.tile([OUT_DIM, fc * P], F32, tag="stage")
                    for i in range(nsp):
                        n0 = i * NSUB
                        n1 = min(n0 + NSUB, NCH)
                        ps = psB.tile([OUT_DIM, NSUB], F32, tag="pso")
                        pss = ps[:, 0:n1 - n0]
                        nc.tensor.matmul(pss, ws["w2a"][:, :],
                                         h1[0][:, n0:n1],
                                         start=True, stop=False)
                        nc.tensor.matmul(pss, ws["w2b"][:, :],
                                         h1[1][:, n0:n1],
                                         start=False, stop=True)
                        nc.scalar.copy(stage[:, n0:n1], pss)
                    nc.sync.dma_start(out=outT[:, g0 * P:(g0 + fci) * P],
                                      in_=stage[:, 0:NCH])
    return nc


_CACHE = {}


def _get_compiled(dt_lerp, has_b0, has_b1, table_elems, fc):
    key = (dt_lerp, has_b0, has_b1, fc)
    if key in _CACHE:
        return _CACHE[key]
    import concourse.bacc as bacc
    nc = bacc.Bacc("TRN2", target_bir_lowering=False,
                   dynamic_dma_scratch_size=int(
                       os.environ.get("DICT_SCRATCH", "16384")))
    build_kernel(nc, dt_lerp, has_b0, has_b1, table_elems, fc)
    nc.compile()
    _CACHE[key] = nc
    return nc


def kernel(coordinates, coeffs, basis_0, basis_1, basis_2, basis_3, basis_4,
           basis_5, w0, b0, w1, b1, w2):
    import ml_dtypes
    from concourse.bass_utils import run_bass_kernel_spmd

    dt_lerp = DT_NAME
    np_dt = np.float32 if dt_lerp == "float32" else ml_dtypes.bfloat16
    fc = FC_DEFAULT

    coordinates = np.ascontiguousarray(np.asarray(coordinates, np.float32))
    packed, table_elems = {}, {}
    pc = _pack_quad(np.asarray(coeffs, np.float32)[0], clamp_border=True)
    pc = _pad_rows_to_256b(pc.astype(np_dt), np_dt)
    packed["coeff"] = pc
    table_elems["coeff"] = (pc.shape[0], pc.shape[1])
    for i, b in enumerate([basis_0, basis_1, basis_2, basis_3, basis_4,
                           basis_5]):
        pb = _pack_quad(np.asarray(b, np.float32)[0], clamp_border=False)
        pb = _pad_rows_to_256b(pb.astype(np_dt), np_dt)
        packed[f"basis{i}"] = pb
        table_elems[f"basis{i}"] = (pb.shape[0], pb.shape[1])

    w0 = np.asarray(w0, np.float32); w1 = np.asarray(w1, np.float32)
    w2 = np.asarray(w2, np.float32)
    b0 = np.asarray(b0, np.float32); b1 = np.asarray(b1, np.float32)
    has_b0 = bool(np.any(b0)); has_b1 = bool(np.any(b1))

    nc = _get_compiled(dt_lerp, has_b0, has_b1, table_elems, fc)

    shared = {
        "w0a": np.ascontiguousarray(w0[0:128]).astype(np_dt),
        "w0b": np.ascontiguousarray(w0[128:144]).astype(np_dt),
        "w1a": np.ascontiguousarray(w1[0:128]).astype(np_dt),
        "w1b": np.ascontiguousarray(w1[128:256]).astype(np_dt),
        "w2a": np.ascontiguousarray(w2[0:128]).astype(np_dt),
        "w2b": np.ascontiguousarray(w2[128:256]).astype(np_dt),
    }
    shared.update(packed)
    if has_b0:
        shared["b0r"] = np.ascontiguousarray(b0.reshape(2, P).T)
    if has_b1:
        shared["b1r"] = np.ascontiguousarray(b1.reshape(2, P).T)

    in_maps = []
    for c in range(N_CORES):
        m = dict(shared)
        m["coords"] = np.ascontiguousarray(coordinates[c * NPC:(c + 1) * NPC])
        in_maps.append(m)

    res = run_bass_kernel_spmd(
        nc, in_maps, core_ids=list(range(N_CORES)),
        trace=bool(int(os.environ.get("DICT_TRACE", "0"))))
    kernel.last_results = res
    # device stores column (g*128 + p) = point (p*230 + g); unscramble here
    outs = []
    for c in range(N_CORES):
        o = res.results[c]["outT"].reshape(OUT_DIM, G, P)
        outs.append(np.transpose(o, (0, 2, 1)).reshape(OUT_DIM, NPC))
    full = np.concatenate(outs, axis=1)                      # (2, N)
    return np.ascontiguousarray(full.T).reshape(IM_H, IM_W, OUT_DIM)
